# revision 6
# baseline (speedup 1.0000x reference)
"""Multi-head attention (B=4, N=1024, C=1024, H=16) on 8 TRN2 NeuronCores.

Sharding: batch B (4-way) x head-group (2-way, 8 heads each).
Core i handles batch b=i//2, head-group hg=i%2.

Per-core outputs: attn_p [Nq,Nk] (sum of the group's 8 scaled score maps)
and y_p [N,C] (projection partial over the group's 512 channels).
Host unshard: pair-sum + bias + /16 head-mean + final transposes.
"""
import numpy as np
import ml_dtypes

import concourse.mybir as mybir
import concourse.tile as tile
from concourse import bacc
from concourse.bass_utils import run_bass_kernel_spmd

P = 128
B, N, C, H = 4, 1024, 1024, 16
HG = 2            # head-group shards
CH = C // HG      # 512 channels per core
NHEAD = H // HG   # 8 heads per core
D = C // H        # 64 head dim
KC = C // P       # 8 contraction chunks over C
CC = CH // P      # 4 chunks over the 512 sharded channels
QT = N // P       # 8 token tiles of 128
QH = N // 512     # 2 token halves of 512
BF = mybir.dt.bfloat16
F32 = mybir.dt.float32

_NC_CACHE = None


def _build_nc():
    nc = bacc.Bacc("TRN2", target_bir_lowering=False, debug=False, num_devices=8)

    xqT = nc.dram_tensor("xqT", [C, N], BF, kind="ExternalInput")
    xkT = nc.dram_tensor("xkT", [C, N], BF, kind="ExternalInput")
    xvT = nc.dram_tensor("xvT", [C, N], BF, kind="ExternalInput")
    wqT = nc.dram_tensor("wqT", [C, CH], BF, kind="ExternalInput")
    wkT = nc.dram_tensor("wkT", [C, CH], BF, kind="ExternalInput")
    wvT = nc.dram_tensor("wvT", [C, CH], BF, kind="ExternalInput")
    wpT = nc.dram_tensor("wpT", [CH, C], BF, kind="ExternalInput")
    attn_p = nc.dram_tensor("attn_p", [N, N], F32, kind="ExternalOutput")
    y_p = nc.dram_tensor("y_p", [N, C], F32, kind="ExternalOutput")

    xqT3 = xqT.rearrange("(ko p) n -> p ko n", p=P)
    xkT3 = xkT.rearrange("(ko p) n -> p ko n", p=P)
    xvT3 = xvT.rearrange("(ko p) n -> p ko n", p=P)
    wqT3 = wqT.rearrange("(ko p) m -> p ko m", p=P)
    wkT3 = wkT.rearrange("(ko p) m -> p ko m", p=P)
    wvT3 = wvT.rearrange("(ko p) m -> p ko m", p=P)
    wpT3 = wpT.rearrange("(ko p) m -> p ko m", p=P)
    attn3 = attn_p.rearrange("(qo p) k -> p qo k", p=P)
    y3 = y_p.rearrange("(to p) c -> p to c", p=P)

    with tile.TileContext(nc) as tc:
        with (
            tc.tile_pool(name="inp", bufs=1) as inp,
            tc.tile_pool(name="w", bufs=1) as wp_,
            tc.tile_pool(name="qkv", bufs=1) as qkv,
            tc.tile_pool(name="pt", bufs=2) as ptp,
            tc.tile_pool(name="sm", bufs=4) as smp,
            tc.tile_pool(name="psA", bufs=3, space="PSUM") as psA,
            tc.tile_pool(name="psS", bufs=3, space="PSUM") as psS,
            tc.tile_pool(name="psO", bufs=2, space="PSUM") as psO,
        ):
            # ---- input / weight loads (chunked for DMA-queue parallelism) ----
            xq_sb = inp.tile([P, KC, N], BF, tag="xq")
            xk_sb = inp.tile([P, KC, N], BF, tag="xk")
            xv_sb = inp.tile([P, KC, N], BF, tag="xv")
            wq_sb = wp_.tile([P, KC, CH], BF, tag="wq")
            wk_sb = wp_.tile([P, KC, CH], BF, tag="wk")
            wv_sb = wp_.tile([P, KC, CH], BF, tag="wv")
            wpp_sb = wp_.tile([P, CC, C], BF, tag="wp")
            for kc in range(KC):
                nc.sync.dma_start(xq_sb[:, kc, :], xqT3[:, kc, :])
                nc.sync.dma_start(wq_sb[:, kc, :], wqT3[:, kc, :])
            for kc in range(KC):
                nc.sync.dma_start(xk_sb[:, kc, :], xkT3[:, kc, :])
                nc.sync.dma_start(wk_sb[:, kc, :], wkT3[:, kc, :])
            for kc in range(KC):
                nc.sync.dma_start(xv_sb[:, kc, :], xvT3[:, kc, :])
                nc.sync.dma_start(wv_sb[:, kc, :], wvT3[:, kc, :])
            for cc in range(CC):
                nc.sync.dma_start(wpp_sb[:, cc, :], wpT3[:, cc, :])

            qt_sb = qkv.tile([P, CC, N], BF, tag="qt")    # Q^T (pre-scaled by 1/8)
            kt_sb = qkv.tile([P, CC, N], BF, tag="kt")    # K^T
            v65_sb = qkv.tile([P, QT, NHEAD * (D + 1)], BF, tag="v65")
            ot_sb = qkv.tile([P, CC, N], BF, tag="ot")    # O^T (normalized)

            # ---- phase A: Q^T, K^T [512ch, N] ----
            for w_sb, x_sb, o_sb, scale in (
                (wq_sb, xq_sb, qt_sb, 0.125),
                (wk_sb, xk_sb, kt_sb, None),
            ):
                for cc in range(CC):
                    pss = [psA.tile([P, 512], F32, tag="pa", name=f"psqk{qh}") for qh in range(QH)]
                    for kc in range(KC):
                        for qh in range(QH):
                            nc.tensor.matmul(
                                pss[qh][:],
                                lhsT=w_sb[:, kc, cc * P:(cc + 1) * P],
                                rhs=x_sb[:, kc, qh * 512:(qh + 1) * 512],
                                start=(kc == 0),
                                stop=(kc == KC - 1),
                            )
                    for qh in range(QH):
                        dst = o_sb[:, cc, qh * 512:(qh + 1) * 512]
                        if scale is not None:
                            nc.scalar.activation(dst, pss[qh][:],
                                                 mybir.ActivationFunctionType.Copy,
                                                 scale=scale)
                        else:
                            nc.vector.tensor_copy(out=dst, in_=pss[qh][:])

            # ---- phase A: V in [tok, head*(64+1)] layout with ones columns ----
            for tt in range(QT):
                pv = psA.tile([P, 512], F32, tag="pa")
                for kc in range(KC):
                    nc.tensor.matmul(
                        pv[:],
                        lhsT=xv_sb[:, kc, tt * P:(tt + 1) * P],
                        rhs=wv_sb[:, kc, :],
                        start=(kc == 0),
                        stop=(kc == KC - 1),
                    )
                for h in range(NHEAD):
                    nc.vector.tensor_copy(
                        out=v65_sb[:, tt, h * (D + 1): h * (D + 1) + D],
                        in_=pv[:, h * D:(h + 1) * D],
                    )
                nc.any.memset(v65_sb[:, tt, D::(D + 1)], 1.0)

            # ---- phase B: attn partial = Q^T . K^T (full 512-ch contraction) ----
            for qt in range(QT):
                pas = [psA.tile([P, 512], F32, tag="pa", name=f"pat{kh}") for kh in range(QH)]
                for cc in range(CC):
                    for kh in range(QH):
                        nc.tensor.matmul(
                            pas[kh][:],
                            lhsT=qt_sb[:, cc, qt * P:(qt + 1) * P],
                            rhs=kt_sb[:, cc, kh * 512:(kh + 1) * 512],
                            start=(cc == 0),
                            stop=(cc == CC - 1),
                        )
                for kh in range(QH):
                    stg = smp.tile([P, 512], F32, tag="stg", name="stga")
                    nc.any.tensor_copy(out=stg[:], in_=pas[kh][:])
                    nc.sync.dma_start(attn3[:, qt, kh * 512:(kh + 1) * 512], stg[:])

            # ---- phase C: per head pair: scores^T -> exp -> PV -> normalize ----
            for hp in range(NHEAD // 2):
                pts = [ptp.tile([P, QT, N], BF, tag=f"pt{h2}", name=f"pt{h2}") for h2 in range(2)]
                # scores + exp (two heads run concurrently on PE via row packing)
                for kt in range(QT):
                    for qh in range(QH):
                        for h2 in range(2):
                            base = 64 * h2
                            ps = psS.tile([P, 512], F32, tag="ps")
                            nc.tensor.matmul(
                                ps[:],
                                lhsT=kt_sb[base:base + 64, hp, kt * P:(kt + 1) * P],
                                rhs=qt_sb[base:base + 64, hp, qh * 512:(qh + 1) * 512],
                                start=True,
                                stop=True,
                            )
                            nc.scalar.activation(
                                pts[h2][:, kt, qh * 512:(qh + 1) * 512], ps[:],
                                mybir.ActivationFunctionType.Exp,
                            )
                # PV with [V | ones] stationary; row 64 of psum = softmax denom
                for h2 in range(2):
                    h = 2 * hp + h2
                    base = 64 * h2
                    for qh in range(QH):
                        po = psO.tile([65, 512], F32, tag="po")
                        for kt in range(QT):
                            nc.tensor.matmul(
                                po[:],
                                lhsT=v65_sb[:, kt, h * (D + 1):(h + 1) * (D + 1)],
                                rhs=pts[h2][:, kt, qh * 512:(qh + 1) * 512],
                                start=(kt == 0),
                                stop=(kt == QT - 1),
                            )
                        recip = smp.tile([1, 512], F32, tag="recip")
                        nc.vector.reciprocal(recip[:], po[64:65, :])
                        bc = smp.tile([64, 512], F32, tag="bc")
                        nc.gpsimd.partition_broadcast(bc[:], recip[:])
                        nc.vector.tensor_mul(
                            out=ot_sb[base:base + 64, hp, qh * 512:(qh + 1) * 512],
                            in0=po[0:64, :],
                            in1=bc[:],
                        )

            # ---- phase D: y partial = O^T^T . Wp^T -> [N, C] ----
            for qt in range(QT):
                pys = [psA.tile([P, 512], F32, tag="pa", name=f"py{ch2}") for ch2 in range(QH)]
                for cc in range(CC):
                    for ch2 in range(QH):
                        nc.tensor.matmul(
                            pys[ch2][:],
                            lhsT=ot_sb[:, cc, qt * P:(qt + 1) * P],
                            rhs=wpp_sb[:, cc, ch2 * 512:(ch2 + 1) * 512],
                            start=(cc == 0),
                            stop=(cc == CC - 1),
                        )
                for ch2 in range(QH):
                    stg = smp.tile([P, 512], F32, tag="stg", name="stgy")
                    nc.any.tensor_copy(out=stg[:], in_=pys[ch2][:])
                    nc.sync.dma_start(y3[:, qt, ch2 * 512:(ch2 + 1) * 512], stg[:])

    nc.compile()
    return nc


def get_nc():
    global _NC_CACHE
    if _NC_CACHE is None:
        _NC_CACHE = _build_nc()
    return _NC_CACHE


def _prep_in_maps(xq, xk, xv, Wq, Wk, Wv, Wp):
    bf = ml_dtypes.bfloat16
    in_maps = []
    xqT = [np.ascontiguousarray(xq[b].T).astype(bf) for b in range(B)]
    xkT = [np.ascontiguousarray(xk[b].T).astype(bf) for b in range(B)]
    xvT = [np.ascontiguousarray(xv[b].T).astype(bf) for b in range(B)]
    wqTs = [np.ascontiguousarray(Wq[hg * CH:(hg + 1) * CH, :].T).astype(bf) for hg in range(HG)]
    wkTs = [np.ascontiguousarray(Wk[hg * CH:(hg + 1) * CH, :].T).astype(bf) for hg in range(HG)]
    wvTs = [np.ascontiguousarray(Wv[hg * CH:(hg + 1) * CH, :].T).astype(bf) for hg in range(HG)]
    wpTs = [np.ascontiguousarray(Wp[:, hg * CH:(hg + 1) * CH].T).astype(bf) for hg in range(HG)]
    for core in range(8):
        b, hg = core // HG, core % HG
        in_maps.append({
            "xqT": xqT[b], "xkT": xkT[b], "xvT": xvT[b],
            "wqT": wqTs[hg], "wkT": wkTs[hg], "wvT": wvTs[hg],
            "wpT": wpTs[hg],
        })
    return in_maps


def kernel(xq, xk, xv, Wq, Wk, Wv, Wp, bp):
    xq, xk, xv = (np.asarray(t, np.float32) for t in (xq, xk, xv))
    Wq, Wk, Wv, Wp, bp = (np.asarray(t, np.float32) for t in (Wq, Wk, Wv, Wp, bp))
    nc = get_nc()
    in_maps = _prep_in_maps(xq, xk, xv, Wq, Wk, Wv, Wp)
    res = run_bass_kernel_spmd(nc, in_maps, list(range(8)))

    x_out = np.empty((B, N, C), np.float32)
    attn = np.empty((B, N, N), np.float32)
    for b in range(B):
        r0, r1 = res.results[2 * b], res.results[2 * b + 1]
        attn[b] = (r0["attn_p"] + r1["attn_p"]) * (1.0 / H)
        x_out[b] = r0["y_p"] + r1["y_p"] + bp
    return x_out.swapaxes(0, 1), attn


# revision 9
# speedup vs baseline: 12082.2047x; 12082.2047x over previous
"""Multi-head attention (B=4, N=1024, C=1024, H=16) on 8 TRN2 NeuronCores.

Sharding: batch B (4-way) x head-group (2-way, 8 heads each).
Core i handles batch b=i//2, head-group hg=i%2.

Per-core outputs: attn_p [Nq,Nk] (sum of the group's 8 scaled score maps)
and y_p [N,C] (projection partial over the group's 512 channels).
Host unshard: pair-sum + bias + /16 head-mean + final transposes.
"""
import numpy as np
import ml_dtypes

import concourse.mybir as mybir
import concourse.tile as tile
from concourse import bacc
from concourse.bass_utils import run_bass_kernel_spmd

P = 128
B, N, C, H = 4, 1024, 1024, 16
HG = 2            # head-group shards
CH = C // HG      # 512 channels per core
NHEAD = H // HG   # 8 heads per core
D = C // H        # 64 head dim
KC = C // P       # 8 contraction chunks over C
CC = CH // P      # 4 chunks over the 512 sharded channels
QT = N // P       # 8 token tiles of 128
QH = N // 512     # 2 token halves of 512
BF = mybir.dt.bfloat16
F32 = mybir.dt.float32

_NC_CACHE = None


def _build_nc(loop_n=1):
    nc = bacc.Bacc("TRN2", target_bir_lowering=False, debug=False, num_devices=8)

    xqT = nc.dram_tensor("xqT", [C, N], BF, kind="ExternalInput")
    xkT = nc.dram_tensor("xkT", [C, N], BF, kind="ExternalInput")
    xvT = nc.dram_tensor("xvT", [C, N], BF, kind="ExternalInput")
    wqT = nc.dram_tensor("wqT", [C, CH], BF, kind="ExternalInput")
    wkT = nc.dram_tensor("wkT", [C, CH], BF, kind="ExternalInput")
    wvT = nc.dram_tensor("wvT", [C, CH], BF, kind="ExternalInput")
    wpT = nc.dram_tensor("wpT", [CH, C], BF, kind="ExternalInput")
    attn_p = nc.dram_tensor("attn_p", [N, N], F32, kind="ExternalOutput")
    y_p = nc.dram_tensor("y_p", [N, C], F32, kind="ExternalOutput")

    xqT3 = xqT.rearrange("(ko p) n -> p ko n", p=P)
    xkT3 = xkT.rearrange("(ko p) n -> p ko n", p=P)
    xvT3 = xvT.rearrange("(ko p) n -> p ko n", p=P)
    wqT3 = wqT.rearrange("(ko p) m -> p ko m", p=P)
    wkT3 = wkT.rearrange("(ko p) m -> p ko m", p=P)
    wvT3 = wvT.rearrange("(ko p) m -> p ko m", p=P)
    wpT3 = wpT.rearrange("(ko p) m -> p ko m", p=P)
    attn3 = attn_p.rearrange("(qo p) k -> p qo k", p=P)
    y3 = y_p.rearrange("(to p) c -> p to c", p=P)

    with tile.TileContext(nc) as tc:
        with (
            tc.tile_pool(name="inp", bufs=1) as inp,
            tc.tile_pool(name="w", bufs=1) as wp_,
            tc.tile_pool(name="qkv", bufs=1) as qkv,
            tc.tile_pool(name="pt", bufs=2) as ptp,
            tc.tile_pool(name="sm", bufs=4) as smp,
            tc.tile_pool(name="psA", bufs=3, space="PSUM") as psA,
            tc.tile_pool(name="psS", bufs=3, space="PSUM") as psS,
            tc.tile_pool(name="psO", bufs=2, space="PSUM") as psO,
        ):
            if loop_n > 1:
                loop_ctx = tc.For_i(0, loop_n, 1)
                loop_ctx.__enter__()
            # ---- input / weight loads (chunked for DMA-queue parallelism) ----
            xq_sb = inp.tile([P, KC, N], BF, tag="xq")
            xk_sb = inp.tile([P, KC, N], BF, tag="xk")
            xv_sb = inp.tile([P, KC, N], BF, tag="xv")
            wq_sb = wp_.tile([P, KC, CH], BF, tag="wq")
            wk_sb = wp_.tile([P, KC, CH], BF, tag="wk")
            wv_sb = wp_.tile([P, KC, CH], BF, tag="wv")
            wpp_sb = wp_.tile([P, CC, C], BF, tag="wp")
            for kc in range(KC):
                nc.sync.dma_start(xq_sb[:, kc, :], xqT3[:, kc, :])
                nc.sync.dma_start(wq_sb[:, kc, :], wqT3[:, kc, :])
            for kc in range(KC):
                nc.sync.dma_start(xk_sb[:, kc, :], xkT3[:, kc, :])
                nc.sync.dma_start(wk_sb[:, kc, :], wkT3[:, kc, :])
            for kc in range(KC):
                nc.sync.dma_start(xv_sb[:, kc, :], xvT3[:, kc, :])
                nc.sync.dma_start(wv_sb[:, kc, :], wvT3[:, kc, :])
            for cc in range(CC):
                nc.sync.dma_start(wpp_sb[:, cc, :], wpT3[:, cc, :])

            qt_sb = qkv.tile([P, CC, N], BF, tag="qt")    # Q^T (pre-scaled by 1/8)
            kt_sb = qkv.tile([P, CC, N], BF, tag="kt")    # K^T
            v65_sb = qkv.tile([P, QT, NHEAD * (D + 1)], BF, tag="v65")
            ot_sb = qkv.tile([P, CC, N], BF, tag="ot")    # O^T (normalized)

            # ---- phase A: Q^T, K^T [512ch, N] ----
            for w_sb, x_sb, o_sb, scale in (
                (wq_sb, xq_sb, qt_sb, 0.125),
                (wk_sb, xk_sb, kt_sb, None),
            ):
                for cc in range(CC):
                    pss = [psA.tile([P, 512], F32, tag="pa", name=f"psqk{qh}") for qh in range(QH)]
                    for kc in range(KC):
                        for qh in range(QH):
                            nc.tensor.matmul(
                                pss[qh][:],
                                lhsT=w_sb[:, kc, cc * P:(cc + 1) * P],
                                rhs=x_sb[:, kc, qh * 512:(qh + 1) * 512],
                                start=(kc == 0),
                                stop=(kc == KC - 1),
                            )
                    for qh in range(QH):
                        dst = o_sb[:, cc, qh * 512:(qh + 1) * 512]
                        if scale is not None:
                            nc.scalar.activation(dst, pss[qh][:],
                                                 mybir.ActivationFunctionType.Copy,
                                                 scale=scale)
                        else:
                            nc.vector.tensor_copy(out=dst, in_=pss[qh][:])

            # ---- phase A: V in [tok, head*(64+1)] layout with ones columns ----
            for tt in range(QT):
                pv = psA.tile([P, 512], F32, tag="pa")
                for kc in range(KC):
                    nc.tensor.matmul(
                        pv[:],
                        lhsT=xv_sb[:, kc, tt * P:(tt + 1) * P],
                        rhs=wv_sb[:, kc, :],
                        start=(kc == 0),
                        stop=(kc == KC - 1),
                    )
                for h in range(NHEAD):
                    nc.vector.tensor_copy(
                        out=v65_sb[:, tt, h * (D + 1): h * (D + 1) + D],
                        in_=pv[:, h * D:(h + 1) * D],
                    )
                nc.any.memset(v65_sb[:, tt, D::(D + 1)], 1.0)

            # ---- phase B: attn partial = Q^T . K^T (full 512-ch contraction) ----
            for qt in range(QT):
                pas = [psA.tile([P, 512], F32, tag="pa", name=f"pat{kh}") for kh in range(QH)]
                for cc in range(CC):
                    for kh in range(QH):
                        nc.tensor.matmul(
                            pas[kh][:],
                            lhsT=qt_sb[:, cc, qt * P:(qt + 1) * P],
                            rhs=kt_sb[:, cc, kh * 512:(kh + 1) * 512],
                            start=(cc == 0),
                            stop=(cc == CC - 1),
                        )
                for kh in range(QH):
                    stg = smp.tile([P, 512], F32, tag="stg", name="stga")
                    nc.any.tensor_copy(out=stg[:], in_=pas[kh][:])
                    nc.sync.dma_start(attn3[:, qt, kh * 512:(kh + 1) * 512], stg[:])

            # ---- phase C: per head pair: scores^T -> exp -> PV -> normalize ----
            for hp in range(NHEAD // 2):
                pts = [ptp.tile([P, QT, N], BF, tag=f"pt{h2}", name=f"pt{h2}") for h2 in range(2)]
                # scores + exp (two heads run concurrently on PE via row packing)
                for kt in range(QT):
                    for qh in range(QH):
                        for h2 in range(2):
                            base = 64 * h2
                            ps = psS.tile([P, 512], F32, tag="ps")
                            nc.tensor.matmul(
                                ps[:],
                                lhsT=kt_sb[base:base + 64, hp, kt * P:(kt + 1) * P],
                                rhs=qt_sb[base:base + 64, hp, qh * 512:(qh + 1) * 512],
                                start=True,
                                stop=True,
                            )
                            nc.scalar.activation(
                                pts[h2][:, kt, qh * 512:(qh + 1) * 512], ps[:],
                                mybir.ActivationFunctionType.Exp,
                            )
                # PV with [V | ones] stationary; row 64 of psum = softmax denom
                for h2 in range(2):
                    h = 2 * hp + h2
                    base = 64 * h2
                    for qh in range(QH):
                        po = psO.tile([65, 512], F32, tag="po")
                        for kt in range(QT):
                            nc.tensor.matmul(
                                po[:],
                                lhsT=v65_sb[:, kt, h * (D + 1):(h + 1) * (D + 1)],
                                rhs=pts[h2][:, kt, qh * 512:(qh + 1) * 512],
                                start=(kt == 0),
                                stop=(kt == QT - 1),
                            )
                        recip = smp.tile([1, 512], F32, tag="recip")
                        nc.vector.reciprocal(recip[:], po[64:65, :])
                        bc = smp.tile([64, 512], F32, tag="bc")
                        nc.gpsimd.partition_broadcast(bc[:], recip[:])
                        nc.vector.tensor_mul(
                            out=ot_sb[base:base + 64, hp, qh * 512:(qh + 1) * 512],
                            in0=po[0:64, :],
                            in1=bc[:],
                        )

            # ---- phase D: y partial = O^T^T . Wp^T -> [N, C] ----
            for qt in range(QT):
                pys = [psA.tile([P, 512], F32, tag="pa", name=f"py{ch2}") for ch2 in range(QH)]
                for cc in range(CC):
                    for ch2 in range(QH):
                        nc.tensor.matmul(
                            pys[ch2][:],
                            lhsT=ot_sb[:, cc, qt * P:(qt + 1) * P],
                            rhs=wpp_sb[:, cc, ch2 * 512:(ch2 + 1) * 512],
                            start=(cc == 0),
                            stop=(cc == CC - 1),
                        )
                for ch2 in range(QH):
                    stg = smp.tile([P, 512], F32, tag="stg", name="stgy")
                    nc.any.tensor_copy(out=stg[:], in_=pys[ch2][:])
                    nc.sync.dma_start(y3[:, qt, ch2 * 512:(ch2 + 1) * 512], stg[:])

            if loop_n > 1:
                loop_ctx.__exit__(None, None, None)

    nc.compile()
    return nc


def get_nc():
    global _NC_CACHE
    if _NC_CACHE is None:
        _NC_CACHE = _build_nc()
    return _NC_CACHE


def _prep_in_maps(xq, xk, xv, Wq, Wk, Wv, Wp):
    bf = ml_dtypes.bfloat16
    in_maps = []
    xqT = [np.ascontiguousarray(xq[b].T).astype(bf) for b in range(B)]
    xkT = [np.ascontiguousarray(xk[b].T).astype(bf) for b in range(B)]
    xvT = [np.ascontiguousarray(xv[b].T).astype(bf) for b in range(B)]
    wqTs = [np.ascontiguousarray(Wq[hg * CH:(hg + 1) * CH, :].T).astype(bf) for hg in range(HG)]
    wkTs = [np.ascontiguousarray(Wk[hg * CH:(hg + 1) * CH, :].T).astype(bf) for hg in range(HG)]
    wvTs = [np.ascontiguousarray(Wv[hg * CH:(hg + 1) * CH, :].T).astype(bf) for hg in range(HG)]
    wpTs = [np.ascontiguousarray(Wp[:, hg * CH:(hg + 1) * CH].T).astype(bf) for hg in range(HG)]
    for core in range(8):
        b, hg = core // HG, core % HG
        in_maps.append({
            "xqT": xqT[b], "xkT": xkT[b], "xvT": xvT[b],
            "wqT": wqTs[hg], "wkT": wkTs[hg], "wvT": wvTs[hg],
            "wpT": wpTs[hg],
        })
    return in_maps


def kernel(xq, xk, xv, Wq, Wk, Wv, Wp, bp):
    xq, xk, xv = (np.asarray(t, np.float32) for t in (xq, xk, xv))
    Wq, Wk, Wv, Wp, bp = (np.asarray(t, np.float32) for t in (Wq, Wk, Wv, Wp, bp))
    nc = get_nc()
    in_maps = _prep_in_maps(xq, xk, xv, Wq, Wk, Wv, Wp)
    res = run_bass_kernel_spmd(nc, in_maps, list(range(8)))

    x_out = np.empty((B, N, C), np.float32)
    attn = np.empty((B, N, N), np.float32)
    for b in range(B):
        r0, r1 = res.results[2 * b], res.results[2 * b + 1]
        attn[b] = (r0["attn_p"] + r1["attn_p"]) * (1.0 / H)
        x_out[b] = r0["y_p"] + r1["y_p"] + bp
    return x_out.swapaxes(0, 1), attn
